# revision 12
# baseline (speedup 1.0000x reference)
"""Trainium2 Bass kernel for a FlowNet-style local correlation layer.

out[b, d, h, w] = (1/C) * sum_c x[b,c,h,w] * ypad[b,c,h+di,w+dj],
d = di*9+dj, displacements in [-4, 4]^2 (K=9, 81 displacements).

Shapes (hardcoded): x, y = [8, 256, 96, 192] fp32 -> out [8, 81, 96, 192] fp32.

Sharding: data-parallel over batch, one batch element per NeuronCore (8 cores).

v5 design (HBM/DMA-bound problem; inputs only need bf16 precision):
  - Host pre-scales both inputs by 1/16 (so products carry the 1/C = 1/256
    scale) and casts to bf16, halving HBM read traffic vs fp32.
  - 16x8 pixel blocks (vs 8x16): the staircase width drops 216 -> 144
    columns/pixel, cutting store traffic by a third (7.96 -> 5.31 MB).
  - Host pre-blocks x into the matmul-stationary layout
    [128p, HB, CH, WB, BH, BW]; each 16-row strip loads as one DMA with
    12 KB contiguous per-partition runs.  y ships as [128p, H, CH, W] so
    strip loads are 12 KB runs too; it lands in a fully resident SBUF tile
    with zeroed top/bottom halo rows.  Columns are NOT padded: out-of-range
    w displacements read wrapped garbage that is mathematically zero in the
    true output; the host zeroes them at extract.
  - Per 16x8 pixel block (144): two accumulating bf16 matmuls
    lhsT = x[c_half, 128 px], rhs = y[c_half, 24x16 region] -> PSUM band
    [128 px, 384].  band[p=(ph,pw), 16*(ph+di) + (pw+dj)] = corr of pixel p
    with displacement (di, dj).
  - PSUM tiles span 4 banks (4 wb-blocks at 512-f32 offsets); one fused copy
    (Vector 2/3, Scalar 1/3) casts 4 blocks to the bf16 band at once,
    reading psum bank-interleaved (cheap) and writing contiguously:
    band col = g*1536 + rcol*4 + k  (wb = 4g + k).
  - Compact staircase store: per (strip, ph row-group) one DMA writes
    SC2=144 columns x4-interleaved (1152 B runs; SWDGE coalesces to 6.9 KB
    dst packets) for all 24 w-blocks; triggers rotate gpsimd (SWDGE) /
    sync / scalar to spread issue cost.
  - Host-side numpy gather assembles the final [81, 96, 192] per element and
    zeroes the w-edge slivers.
"""

import sys

for _p in ("/opt/trn_rl_repo", "/root/.axon_site/_ro/trn_rl_repo"):
    if _p not in sys.path:
        sys.path.insert(0, _p)

import ml_dtypes
import numpy as np

import concourse.bass as bass
import concourse.mybir as mybir
import concourse.tile as tile
from concourse import bacc
from concourse.bass_utils import run_bass_kernel_spmd

# Problem constants (hardcoded per spec)
B, C, H, W = 8, 256, 96, 192
MD = 4
K = 2 * MD + 1          # 9
D = K * K               # 81
BH, BW = 16, 8          # pixel block = 16 rows x 8 cols = 128 pixels
HB, WB = H // BH, W // BW   # 6 strips x 24 col-blocks = 144 blocks
RH, RW = BH + 2 * MD, BW + 2 * MD   # region 24 x 16
NB = RH * RW            # 384 psum band columns
SC2 = RW * (K - 1) + BW + K - 1     # staircase width: 16*8 + 8 + 8 = 144
NG = WB // 4            # 6 wb-groups per strip
CH = C // 128           # 2 contraction halves
PT = 5                  # top zero rows in y_sb (4 halo + 1 offset guard)
HPP = PT + H + 5        # y_sb rows: 5 + 96 + 4 halo + 1 wrap guard = 106
BROW = WB * NB          # band row: 24*384 = 9216 (staircase fits exactly)

F32 = mybir.dt.float32
BF16 = mybir.dt.bfloat16
BF16_NP = ml_dtypes.bfloat16

_CACHE = {}


def _build_nc(n_cores: int):
    nc = bacc.Bacc(
        "TRN2",
        target_bir_lowering=False,
        debug=False,
        enable_asserts=False,
        num_devices=n_cores,
    )
    # partition-major host layouts: c = ch*128 + p
    x_d = nc.dram_tensor("x", [128, HB, CH, WB, BH, BW], BF16, kind="ExternalInput")
    y_d = nc.dram_tensor("y", [128, H, CH, W], BF16, kind="ExternalInput")
    # out[hb, ph, pw, q, rcol', k] with wb = 4*q + k, rcol' = 16*di + pw + dj
    o_d = nc.dram_tensor("out", [HB, BH, BW, NG, SC2, 4], BF16, kind="ExternalOutput")

    with tile.TileContext(nc) as tc:
        with (
            tc.tile_pool(name="big", bufs=1) as big,
            tc.tile_pool(name="xblk", bufs=3) as xblkp,
            tc.tile_pool(name="band", bufs=3) as bandp,
            tc.tile_pool(name="ps", bufs=2, space="PSUM") as psump,
        ):
            y_sb = big.tile([128, HPP, CH, W], BF16)

            # zero the y top/bottom halo rows once (no w pad: w edges are
            # garbage on device, zeroed on host); split across engines
            nc.vector.memset(y_sb[:, 0:PT], 0.0)
            nc.gpsimd.memset(y_sb[:, PT + H : HPP], 0.0)

            xblks = {}

            def issue_y(hb):
                if hb >= HB:
                    return
                r0 = hb * BH
                nc.sync.dma_start(
                    y_sb[:, PT + r0 : PT + r0 + BH],
                    y_d[:, r0 : r0 + BH],
                )

            def issue_x(hb):
                if hb >= HB:
                    return
                xb = xblkp.tile([128, CH, WB, BH, BW], BF16)
                xblks[hb] = xb
                nc.sync.dma_start(xb[:, :], x_d[:, hb])

            # interleave initial loads so strip 0 is ready ASAP
            issue_y(0)
            issue_x(0)
            issue_y(1)
            issue_x(1)
            issue_y(2)

            for hb in range(HB):
                issue_y(hb + 3)
                issue_x(hb + 2)
                xb = xblks.pop(hb)
                band = bandp.tile([128, BROW], BF16)
                # NG groups of 4 wb-blocks; each group fills one 4-bank PSUM
                # tile (4 x 384 f32 at 512-aligned offsets), then one fused
                # copy moves all 4 bands to SBUF.
                for g in range(NG):
                    ps = psump.tile([128, 4 * 512], F32)
                    for k in range(4):
                        wb = g * 4 + k
                        w0 = wb * BW
                        psk = ps[:, k * 512 : k * 512 + NB]
                        for ch in range(CH):
                            # region rows: y rows 16hb-4 .. 16hb+19 -> y_sb
                            # rows (PT-4)+16hb .. ; cols w0-4 .. w0+11 (may
                            # wrap at w edges -> garbage, zeroed on host)
                            src = bass.AP(
                                y_sb.tensor,
                                y_sb.offset
                                + (PT - MD + hb * BH) * CH * W
                                + ch * W
                                + w0 - MD,
                                [[HPP * CH * W, 128], [CH * W, RH], [1, RW]],
                            )
                            nc.tensor.matmul(
                                psk,
                                xb[:, ch, wb],
                                src,
                                start=(ch == 0),
                                stop=(ch == CH - 1),
                            )
                    # fused 4-way-interleaved band write:
                    # band col = g*1536 + rcol*4 + k (wb = 4g + k).  The
                    # engine READS psum bank-interleaved (cheap) and WRITES
                    # the band contiguously, so staircase runs are 4x longer.
                    srcv = bass.AP(
                        ps.tensor,
                        ps.offset,
                        [[4 * 512, 128], [1, NB], [512, 4]],
                    )
                    dstv = band[:, g * 4 * NB : (g + 1) * 4 * NB]
                    if g % 3 != 2:
                        nc.vector.tensor_copy(dstv, srcv)
                    else:
                        nc.scalar.activation(
                            dstv, srcv, mybir.ActivationFunctionType.Copy
                        )
                # compact staircase store: row-group ph covers band columns
                # [4*16*ph, 4*(16*ph + SC2)) of every wb-group; triggers
                # rotate gpsimd (SWDGE) / sync / scalar
                for ph in range(BH):
                    src = bass.AP(
                        band.tensor,
                        band.offset + (BW * ph) * BROW + 4 * RW * ph,
                        [[BROW, BW], [4 * NB, NG], [1, 4 * SC2]],
                    )
                    eng = (nc.gpsimd, nc.sync, nc.gpsimd, nc.scalar)[ph % 4]
                    eng.dma_start(o_d[hb, ph], src)

    nc.compile()
    return nc


def _get_nc():
    if "nc" not in _CACHE:
        _CACHE["nc"] = _build_nc(B)
    return _CACHE["nc"]


def host_extract(stored: np.ndarray) -> np.ndarray:
    """stored: [B, HB, BH, BW, NG, SC2, 4] -> out [B, D, H, W] float32.

    out[b, (di,dj), hb*16+ph, (4q+k)*8+pw] =
        stored[b, hb, ph, pw, q, 16*di+pw+dj, k]
    then w-edge slivers (out-of-range dj) are zeroed.
    """
    st = np.asarray(stored, dtype=np.float32)
    di = np.arange(K).reshape(K, 1, 1)
    dj = np.arange(K).reshape(1, K, 1)
    pw = np.arange(BW).reshape(1, 1, BW)
    col = RW * di + pw + dj                      # (K, K, BW)
    pw_idx = np.broadcast_to(pw, (K, K, BW))     # (K, K, BW)
    # advanced indices at axes 3 and 5 are non-adjacent -> result dims are
    # (K, K, BW, B, HB, BH, NG, 4)
    g = st[:, :, :, pw_idx, :, col, :]
    out = g.transpose(3, 0, 1, 4, 5, 6, 7, 2)    # [B, K, K, HB, BH, NG, 4, BW]
    out = np.ascontiguousarray(out.reshape(B, D, H, W))
    # zero the w-edge slivers: displacement o = dj - MD out of range
    ov = out.reshape(B, K, K, H, W)
    for dj_ in range(K):
        o = dj_ - MD
        if o < 0:
            ov[:, :, dj_, :, 0:-o] = 0.0
        elif o > 0:
            ov[:, :, dj_, :, W - o : W] = 0.0
    return out


def kernel(x, y, max_displacement=MD):
    assert int(max_displacement) == MD
    x = np.asarray(x, dtype=np.float32)
    y = np.asarray(y, dtype=np.float32)
    assert x.shape == (B, C, H, W) and y.shape == (B, C, H, W)

    # fold the 1/C = 1/256 output scale into the inputs (1/16 each; exact in
    # bf16) and cast to bf16 to halve HBM read traffic.
    xs = (x * (1.0 / 16.0)).astype(BF16_NP)
    ys = (y * (1.0 / 16.0)).astype(BF16_NP)
    # partition-major blocked layouts (c = ch*128 + p):
    # x: [B,C,H,W] -> [128, B, HB, CH, WB, BH, BW]
    xs = xs.reshape(B, CH, 128, HB, BH, WB, BW).transpose(2, 0, 3, 1, 5, 4, 6)
    # y: [B,C,H,W] -> [128, B, H, CH, W]
    ys = ys.reshape(B, CH, 128, H, W).transpose(2, 0, 3, 1, 4)

    nc = _get_nc()
    in_maps = [
        {"x": np.ascontiguousarray(xs[:, b]), "y": np.ascontiguousarray(ys[:, b])}
        for b in range(B)
    ]
    res = run_bass_kernel_spmd(nc, in_maps, core_ids=list(range(B)))
    stored = np.stack([r["out"] for r in res.results])  # [B,HB,BH,BW,NG,SC2,4]
    return host_extract(stored)


if __name__ == "__main__":
    rng = np.random.default_rng(0)
    x = rng.standard_normal((B, C, H, W), dtype=np.float32)
    y = rng.standard_normal((B, C, H, W), dtype=np.float32)
    out = kernel(x=x, y=y, max_displacement=4)
    print("kernel ran, out shape", out.shape, out.dtype)


# revision 13
# speedup vs baseline: 1.0341x; 1.0341x over previous
"""Trainium2 Bass kernel for a FlowNet-style local correlation layer.

out[b, d, h, w] = (1/C) * sum_c x[b,c,h,w] * ypad[b,c,h+di,w+dj],
d = di*9+dj, displacements in [-4, 4]^2 (K=9, 81 displacements).

Shapes (hardcoded): x, y = [8, 256, 96, 192] fp32 -> out [8, 81, 96, 192] fp32.

Sharding: data-parallel over batch, one batch element per NeuronCore (8 cores).

v6 design (HBM/DMA-bound problem; inputs only need bf16 precision):
  - Host pre-scales both inputs by 1/16 (so products carry the 1/C = 1/256
    scale) and casts to bf16, halving HBM read traffic vs fp32.
  - Host pre-blocks x into the matmul-stationary layout
    [128p, HB, CH, WB, BH, BW] (6 KB contiguous per-partition strip runs).
    y ships as [128p, H, CH, W] so 8-row strip loads are 6 KB runs into a
    fully resident SBUF tile with zeroed top/bottom halo rows.  Columns are
    NOT padded: out-of-range w displacements read wrapped garbage that is
    mathematically zero in the true output; the host zeroes them at extract.
  - Per 8x16 pixel block (144): two accumulating bf16 matmuls
    lhsT = x[c_half, 128 px], rhs = y[c_half, 16x24 region] -> PSUM band
    [128 px, 384].  band[p=(ph,pw), 24*(ph+di) + (pw+dj)] = corr of pixel p
    with displacement (di, dj).
  - PSUM tiles span 2 banks (2 wb-blocks at 512-f32 offsets), pool depth 4,
    so copies have 3 groups of slack and never stall the tensor engine.
    Each fused copy (Vector/Scalar alternating) reads psum bank-interleaved
    (cheap) and writes the band contiguously:
    band col = t*768 + rcol*2 + k  (wb = 2t + k).
  - Compact staircase store: per (strip, ph row-group) one DMA writes
    SC2=216 columns x2-interleaved (864 B runs; SWDGE coalesces to 5 KB dst
    packets) for all 12 w-blocks; triggers mostly on the otherwise-idle
    gpsimd (SWDGE), rest on sync/scalar.
  - Host-side numpy gather assembles the final [81, 96, 192] per element and
    zeroes the w-edge slivers.
"""

import sys

for _p in ("/opt/trn_rl_repo", "/root/.axon_site/_ro/trn_rl_repo"):
    if _p not in sys.path:
        sys.path.insert(0, _p)

import ml_dtypes
import numpy as np

import concourse.bass as bass
import concourse.mybir as mybir
import concourse.tile as tile
from concourse import bacc
from concourse.bass_utils import run_bass_kernel_spmd

# Problem constants (hardcoded per spec)
B, C, H, W = 8, 256, 96, 192
MD = 4
K = 2 * MD + 1          # 9
D = K * K               # 81
BH, BW = 8, 16          # pixel block = 8 rows x 16 cols = 128 pixels
HB, WB = H // BH, W // BW   # 12 strips x 12 col-blocks = 144 blocks
RH, RW = BH + 2 * MD, BW + 2 * MD   # region 16 x 24
NB = RH * RW            # 384 psum band columns
SC2 = RW * (K - 1) + BW + K - 1     # staircase width: 24*8 + 16 + 8 = 216
NT = WB // 2            # 6 wb-pairs per strip
CH = C // 128           # 2 contraction halves
PT = 5                  # top zero rows in y_sb (4 halo + 1 offset guard)
HPP = PT + H + 5        # y_sb rows: 5 + 96 + 4 halo + 1 wrap guard = 106
BROW = WB * NB          # band row: 12*384 = 4608 (staircase fits exactly)

F32 = mybir.dt.float32
BF16 = mybir.dt.bfloat16
BF16_NP = ml_dtypes.bfloat16

_CACHE = {}


def _build_nc(n_cores: int):
    nc = bacc.Bacc(
        "TRN2",
        target_bir_lowering=False,
        debug=False,
        enable_asserts=False,
        num_devices=n_cores,
    )
    # partition-major host layouts: c = ch*128 + p
    x_d = nc.dram_tensor("x", [128, HB, CH, WB, BH, BW], BF16, kind="ExternalInput")
    y_d = nc.dram_tensor("y", [128, H, CH, W], BF16, kind="ExternalInput")
    # out[hb, ph, pw, t, rcol', k] with wb = 2*t + k, rcol' = 24*di + pw + dj
    o_d = nc.dram_tensor("out", [HB, BH, BW, NT, SC2, 2], BF16, kind="ExternalOutput")

    with tile.TileContext(nc) as tc:
        with (
            tc.tile_pool(name="big", bufs=1) as big,
            tc.tile_pool(name="xblk", bufs=4) as xblkp,
            tc.tile_pool(name="band", bufs=3) as bandp,
            tc.tile_pool(name="ps", bufs=4, space="PSUM") as psump,
        ):
            y_sb = big.tile([128, HPP, CH, W], BF16)

            # zero the y top/bottom halo rows once (no w pad: w edges are
            # garbage on device, zeroed on host); split across engines
            nc.vector.memset(y_sb[:, 0:PT], 0.0)
            nc.gpsimd.memset(y_sb[:, PT + H : HPP], 0.0)

            xblks = {}

            def issue_y(hb):
                if hb >= HB:
                    return
                r0 = hb * BH
                nc.sync.dma_start(
                    y_sb[:, PT + r0 : PT + r0 + BH],
                    y_d[:, r0 : r0 + BH],
                )

            def issue_x(hb):
                if hb >= HB:
                    return
                xb = xblkp.tile([128, CH, WB, BH, BW], BF16)
                xblks[hb] = xb
                nc.sync.dma_start(xb[:, :], x_d[:, hb])

            # interleave initial loads so strip 0 is ready ASAP
            issue_y(0)
            issue_x(0)
            issue_y(1)
            issue_x(1)
            issue_y(2)

            for hb in range(HB):
                issue_y(hb + 3)
                issue_x(hb + 2)
                xb = xblks.pop(hb)
                band = bandp.tile([128, BROW], BF16)
                # NT pairs of wb-blocks; each pair fills one 2-bank PSUM
                # tile (2 x 384 f32 at 512-aligned offsets), then one fused
                # copy moves both bands to SBUF.
                for t in range(NT):
                    ps = psump.tile([128, 2 * 512], F32)
                    for k in range(2):
                        wb = t * 2 + k
                        w0 = wb * BW
                        psk = ps[:, k * 512 : k * 512 + NB]
                        for ch in range(CH):
                            # region rows: y rows 8hb-4 .. 8hb+11 -> y_sb rows
                            # (PT-4)+8hb .. ; cols w0-4 .. w0+19 (may wrap
                            # at w edges -> garbage, zeroed on host)
                            src = bass.AP(
                                y_sb.tensor,
                                y_sb.offset
                                + (PT - MD + hb * BH) * CH * W
                                + ch * W
                                + w0 - MD,
                                [[HPP * CH * W, 128], [CH * W, RH], [1, RW]],
                            )
                            nc.tensor.matmul(
                                psk,
                                xb[:, ch, wb],
                                src,
                                start=(ch == 0),
                                stop=(ch == CH - 1),
                            )
                    # fused 2-way-interleaved band write:
                    # band col = t*768 + rcol*2 + k (wb = 2t + k).  The
                    # engine READS psum bank-interleaved (cheap) and WRITES
                    # the band contiguously, doubling staircase run length.
                    srcv = bass.AP(
                        ps.tensor,
                        ps.offset,
                        [[2 * 512, 128], [1, NB], [512, 2]],
                    )
                    dstv = band[:, t * 2 * NB : (t + 1) * 2 * NB]
                    if t % 2 == 0:
                        nc.vector.tensor_copy(dstv, srcv)
                    else:
                        nc.scalar.activation(
                            dstv, srcv, mybir.ActivationFunctionType.Copy
                        )
                # compact staircase store: row-group ph covers band columns
                # [2*24*ph, 2*(24*ph + SC2)) of every wb-pair; triggers
                # mostly gpsimd (SWDGE), rest sync / scalar
                for ph in range(BH):
                    src = bass.AP(
                        band.tensor,
                        band.offset + (BW * ph) * BROW + 2 * RW * ph,
                        [[BROW, BW], [2 * NB, NT], [1, 2 * SC2]],
                    )
                    eng = (nc.gpsimd, nc.sync, nc.gpsimd, nc.scalar,
                           nc.gpsimd, nc.sync, nc.gpsimd, nc.gpsimd)[ph]
                    eng.dma_start(o_d[hb, ph], src)

    nc.compile()
    return nc


def _get_nc():
    if "nc" not in _CACHE:
        _CACHE["nc"] = _build_nc(B)
    return _CACHE["nc"]


def host_extract(stored: np.ndarray) -> np.ndarray:
    """stored: [B, HB, BH, BW, NT, SC2, 2] -> out [B, D, H, W] float32.

    out[b, (di,dj), hb*8+ph, (2t+k)*16+pw] =
        stored[b, hb, ph, pw, t, 24*di+pw+dj, k]
    then w-edge slivers (out-of-range dj) are zeroed.
    """
    st = np.asarray(stored, dtype=np.float32)
    di = np.arange(K).reshape(K, 1, 1)
    dj = np.arange(K).reshape(1, K, 1)
    pw = np.arange(BW).reshape(1, 1, BW)
    col = RW * di + pw + dj                      # (K, K, BW)
    pw_idx = np.broadcast_to(pw, (K, K, BW))     # (K, K, BW)
    # advanced indices at axes 3 and 5 are non-adjacent -> result dims are
    # (K, K, BW, B, HB, BH, NT, 2)
    g = st[:, :, :, pw_idx, :, col, :]
    out = g.transpose(3, 0, 1, 4, 5, 6, 7, 2)    # [B, K, K, HB, BH, NT, 2, BW]
    out = np.ascontiguousarray(out.reshape(B, D, H, W))
    # zero the w-edge slivers: displacement o = dj - MD out of range
    ov = out.reshape(B, K, K, H, W)
    for dj_ in range(K):
        o = dj_ - MD
        if o < 0:
            ov[:, :, dj_, :, 0:-o] = 0.0
        elif o > 0:
            ov[:, :, dj_, :, W - o : W] = 0.0
    return out


def kernel(x, y, max_displacement=MD):
    assert int(max_displacement) == MD
    x = np.asarray(x, dtype=np.float32)
    y = np.asarray(y, dtype=np.float32)
    assert x.shape == (B, C, H, W) and y.shape == (B, C, H, W)

    # fold the 1/C = 1/256 output scale into the inputs (1/16 each; exact in
    # bf16) and cast to bf16 to halve HBM read traffic.
    xs = (x * (1.0 / 16.0)).astype(BF16_NP)
    ys = (y * (1.0 / 16.0)).astype(BF16_NP)
    # partition-major blocked layouts (c = ch*128 + p):
    # x: [B,C,H,W] -> [128, B, HB, CH, WB, BH, BW]
    xs = xs.reshape(B, CH, 128, HB, BH, WB, BW).transpose(2, 0, 3, 1, 5, 4, 6)
    # y: [B,C,H,W] -> [128, B, H, CH, W]
    ys = ys.reshape(B, CH, 128, H, W).transpose(2, 0, 3, 1, 4)

    nc = _get_nc()
    in_maps = [
        {"x": np.ascontiguousarray(xs[:, b]), "y": np.ascontiguousarray(ys[:, b])}
        for b in range(B)
    ]
    res = run_bass_kernel_spmd(nc, in_maps, core_ids=list(range(B)))
    stored = np.stack([r["out"] for r in res.results])  # [B,HB,BH,BW,NT,SC2,2]
    return host_extract(stored)


if __name__ == "__main__":
    rng = np.random.default_rng(0)
    x = rng.standard_normal((B, C, H, W), dtype=np.float32)
    y = rng.standard_normal((B, C, H, W), dtype=np.float32)
    out = kernel(x=x, y=y, max_displacement=4)
    print("kernel ran, out shape", out.shape, out.dtype)


# revision 14
# speedup vs baseline: 1.0368x; 1.0026x over previous
"""Trainium2 Bass kernel for a FlowNet-style local correlation layer.

out[b, d, h, w] = (1/C) * sum_c x[b,c,h,w] * ypad[b,c,h+di,w+dj],
d = di*9+dj, displacements in [-4, 4]^2 (K=9, 81 displacements).

Shapes (hardcoded): x, y = [8, 256, 96, 192] fp32 -> out [8, 81, 96, 192] fp32.

Sharding: data-parallel over batch, one batch element per NeuronCore (8 cores).

v7 design (DMA-byte-bound; inputs only need bf16 precision):
  - Host pre-scales both inputs by 1/16 (so products carry the 1/C = 1/256
    scale) and casts to bf16, halving HBM read traffic vs fp32.
  - 16x8 pixel blocks: staircase width 144 cols/pixel (vs 216 for 8x16),
    cutting store traffic by a third (7.96 -> 5.31 MB).
  - Host pre-blocks x into the matmul-stationary layout
    [128p, HB, CH, WB, BH, BW]; each 16-row block-strip loads as one DMA
    with 12 KB contiguous per-partition runs.  y ships as [128p, H, CH, W]
    and loads in fine-grained 8-row chunks (6 KB runs) to keep the pipeline
    ramp short; it is fully resident with zeroed top/bottom halo rows.
    Columns are NOT padded: out-of-range w displacements read wrapped
    garbage that is mathematically zero; the host zeroes them at extract.
  - Per 16x8 pixel block (144): two accumulating bf16 matmuls
    lhsT = x[c_half, 128 px], rhs = y[c_half, 24x16 region] -> PSUM band
    [128 px, 384].  band[p=(ph,pw), 16*(ph+di) + (pw+dj)] = corr of pixel p
    with displacement (di, dj).
  - PSUM tiles span 2 banks (2 wb-blocks at 512-f32 offsets), pool depth 4,
    so copies have slack and do not stall the tensor engine.  Each fused
    copy (Vector 7/12, Scalar 5/12) reads psum bank-interleaved (cheap) and
    writes the band contiguously: band col = t*768 + rcol*2 + k (wb = 2t+k).
  - Compact staircase store: per (block-strip, ph row-group) one DMA writes
    SC2=144 columns x2-interleaved (576 B runs; SWDGE coalesces dst-side)
    for all 24 w-blocks; triggers mostly on gpsimd (SWDGE), rest sync/scalar.
  - Host-side numpy gather assembles the final [81, 96, 192] per element and
    zeroes the w-edge slivers.
"""

import sys

for _p in ("/opt/trn_rl_repo", "/root/.axon_site/_ro/trn_rl_repo"):
    if _p not in sys.path:
        sys.path.insert(0, _p)

import ml_dtypes
import numpy as np

import concourse.bass as bass
import concourse.mybir as mybir
import concourse.tile as tile
from concourse import bacc
from concourse.bass_utils import run_bass_kernel_spmd

# Problem constants (hardcoded per spec)
B, C, H, W = 8, 256, 96, 192
MD = 4
K = 2 * MD + 1          # 9
D = K * K               # 81
BH, BW = 16, 8          # pixel block = 16 rows x 8 cols = 128 pixels
HB, WB = H // BH, W // BW   # 6 block-strips x 24 col-blocks = 144 blocks
RH, RW = BH + 2 * MD, BW + 2 * MD   # region 24 x 16
NB = RH * RW            # 384 psum band columns
SC2 = RW * (K - 1) + BW + K - 1     # staircase width: 16*8 + 8 + 8 = 144
NT = WB // 2            # 12 wb-pairs per strip
CH = C // 128           # 2 contraction halves
PT = 5                  # top zero rows in y_sb (4 halo + 1 offset guard)
HPP = PT + H + 5        # y_sb rows: 5 + 96 + 4 halo + 1 wrap guard = 106
BROW = WB * NB          # band row: 24*384 = 9216 (staircase fits exactly)
YC = H // 8             # 12 y load chunks of 8 rows

F32 = mybir.dt.float32
BF16 = mybir.dt.bfloat16
BF16_NP = ml_dtypes.bfloat16

_CACHE = {}


def _build_nc(n_cores: int):
    nc = bacc.Bacc(
        "TRN2",
        target_bir_lowering=False,
        debug=False,
        enable_asserts=False,
        num_devices=n_cores,
    )
    # partition-major host layouts: c = ch*128 + p
    x_d = nc.dram_tensor("x", [128, HB, CH, WB, BH, BW], BF16, kind="ExternalInput")
    y_d = nc.dram_tensor("y", [128, H, CH, W], BF16, kind="ExternalInput")
    # out[hb, ph, pw, t, rcol', k] with wb = 2*t + k, rcol' = 16*di + pw + dj
    o_d = nc.dram_tensor("out", [HB, BH, BW, NT, SC2, 2], BF16, kind="ExternalOutput")

    with tile.TileContext(nc) as tc:
        with (
            tc.tile_pool(name="big", bufs=1) as big,
            tc.tile_pool(name="xblk", bufs=3) as xblkp,
            tc.tile_pool(name="band", bufs=3) as bandp,
            tc.tile_pool(name="ps", bufs=4, space="PSUM") as psump,
        ):
            y_sb = big.tile([128, HPP, CH, W], BF16)

            # zero the y top/bottom halo rows once (no w pad: w edges are
            # garbage on device, zeroed on host); split across engines
            nc.vector.memset(y_sb[:, 0:PT], 0.0)
            nc.gpsimd.memset(y_sb[:, PT + H : HPP], 0.0)

            xblks = {}

            def issue_y(c):
                # 8-row y chunk c
                if c >= YC:
                    return
                r0 = c * 8
                nc.sync.dma_start(
                    y_sb[:, PT + r0 : PT + r0 + 8],
                    y_d[:, r0 : r0 + 8],
                )

            def issue_x(hb):
                if hb >= HB:
                    return
                xb = xblkp.tile([128, CH, WB, BH, BW], BF16)
                xblks[hb] = xb
                nc.sync.dma_start(xb[:, :], x_d[:, hb])

            # block-strip hb consumes y chunks 2hb-1 .. 2hb+2 (halo rows).
            # interleave initial loads so strip 0 is ready ASAP
            issue_y(0)
            issue_y(1)
            issue_x(0)
            issue_y(2)
            issue_y(3)
            issue_x(1)
            issue_y(4)

            # copy engine per wb-pair: Vector 7, Scalar 5
            cp_eng = [0, 1, 0, 1, 0, 1, 0, 1, 0, 1, 0, 0]
            # store trigger engine per ph: gpsimd 10, sync 4, scalar 2
            st_eng = [0, 1, 0, 2, 0, 1, 0, 0, 0, 1, 0, 2, 0, 1, 0, 0]

            for hb in range(HB):
                issue_y(2 * hb + 5)
                issue_y(2 * hb + 6)
                issue_x(hb + 2)
                xb = xblks.pop(hb)
                band = bandp.tile([128, BROW], BF16)
                # NT pairs of wb-blocks; each pair fills one 2-bank PSUM
                # tile (2 x 384 f32 at 512-aligned offsets), then one fused
                # copy moves both bands to SBUF.
                for t in range(NT):
                    ps = psump.tile([128, 2 * 512], F32)
                    for k in range(2):
                        wb = t * 2 + k
                        w0 = wb * BW
                        psk = ps[:, k * 512 : k * 512 + NB]
                        for ch in range(CH):
                            # region rows: y rows 16hb-4 .. 16hb+19 -> y_sb
                            # rows (PT-4)+16hb .. ; cols w0-4 .. w0+11 (may
                            # wrap at w edges -> garbage, zeroed on host)
                            src = bass.AP(
                                y_sb.tensor,
                                y_sb.offset
                                + (PT - MD + hb * BH) * CH * W
                                + ch * W
                                + w0 - MD,
                                [[HPP * CH * W, 128], [CH * W, RH], [1, RW]],
                            )
                            nc.tensor.matmul(
                                psk,
                                xb[:, ch, wb],
                                src,
                                start=(ch == 0),
                                stop=(ch == CH - 1),
                            )
                    # fused 2-way-interleaved band write:
                    # band col = t*768 + rcol*2 + k (wb = 2t + k).  The
                    # engine READS psum bank-interleaved (cheap) and WRITES
                    # the band contiguously, doubling staircase run length.
                    srcv = bass.AP(
                        ps.tensor,
                        ps.offset,
                        [[2 * 512, 128], [1, NB], [512, 2]],
                    )
                    dstv = band[:, t * 2 * NB : (t + 1) * 2 * NB]
                    if cp_eng[t] == 0:
                        nc.vector.tensor_copy(dstv, srcv)
                    else:
                        nc.scalar.activation(
                            dstv, srcv, mybir.ActivationFunctionType.Copy
                        )
                # compact staircase store: row-group ph covers band columns
                # [2*16*ph, 2*(16*ph + SC2)) of every wb-pair; triggers
                # mostly gpsimd (SWDGE), rest sync / scalar
                for ph in range(BH):
                    src = bass.AP(
                        band.tensor,
                        band.offset + (BW * ph) * BROW + 2 * RW * ph,
                        [[BROW, BW], [2 * NB, NT], [1, 2 * SC2]],
                    )
                    eng = (nc.gpsimd, nc.sync, nc.scalar)[st_eng[ph]]
                    eng.dma_start(o_d[hb, ph], src)

    nc.compile()
    return nc


def _get_nc():
    if "nc" not in _CACHE:
        _CACHE["nc"] = _build_nc(B)
    return _CACHE["nc"]


def host_extract(stored: np.ndarray) -> np.ndarray:
    """stored: [B, HB, BH, BW, NT, SC2, 2] -> out [B, D, H, W] float32.

    out[b, (di,dj), hb*16+ph, (2t+k)*8+pw] =
        stored[b, hb, ph, pw, t, 16*di+pw+dj, k]
    then w-edge slivers (out-of-range dj) are zeroed.
    """
    st = np.asarray(stored, dtype=np.float32)
    di = np.arange(K).reshape(K, 1, 1)
    dj = np.arange(K).reshape(1, K, 1)
    pw = np.arange(BW).reshape(1, 1, BW)
    col = RW * di + pw + dj                      # (K, K, BW)
    pw_idx = np.broadcast_to(pw, (K, K, BW))     # (K, K, BW)
    # advanced indices at axes 3 and 5 are non-adjacent -> result dims are
    # (K, K, BW, B, HB, BH, NT, 2)
    g = st[:, :, :, pw_idx, :, col, :]
    out = g.transpose(3, 0, 1, 4, 5, 6, 7, 2)    # [B, K, K, HB, BH, NT, 2, BW]
    out = np.ascontiguousarray(out.reshape(B, D, H, W))
    # zero the w-edge slivers: displacement o = dj - MD out of range
    ov = out.reshape(B, K, K, H, W)
    for dj_ in range(K):
        o = dj_ - MD
        if o < 0:
            ov[:, :, dj_, :, 0:-o] = 0.0
        elif o > 0:
            ov[:, :, dj_, :, W - o : W] = 0.0
    return out


def kernel(x, y, max_displacement=MD):
    assert int(max_displacement) == MD
    x = np.asarray(x, dtype=np.float32)
    y = np.asarray(y, dtype=np.float32)
    assert x.shape == (B, C, H, W) and y.shape == (B, C, H, W)

    # fold the 1/C = 1/256 output scale into the inputs (1/16 each; exact in
    # bf16) and cast to bf16 to halve HBM read traffic.
    xs = (x * (1.0 / 16.0)).astype(BF16_NP)
    ys = (y * (1.0 / 16.0)).astype(BF16_NP)
    # partition-major blocked layouts (c = ch*128 + p):
    # x: [B,C,H,W] -> [128, B, HB, CH, WB, BH, BW]
    xs = xs.reshape(B, CH, 128, HB, BH, WB, BW).transpose(2, 0, 3, 1, 5, 4, 6)
    # y: [B,C,H,W] -> [128, B, H, CH, W]
    ys = ys.reshape(B, CH, 128, H, W).transpose(2, 0, 3, 1, 4)

    nc = _get_nc()
    in_maps = [
        {"x": np.ascontiguousarray(xs[:, b]), "y": np.ascontiguousarray(ys[:, b])}
        for b in range(B)
    ]
    res = run_bass_kernel_spmd(nc, in_maps, core_ids=list(range(B)))
    stored = np.stack([r["out"] for r in res.results])  # [B,HB,BH,BW,NT,SC2,2]
    return host_extract(stored)


if __name__ == "__main__":
    rng = np.random.default_rng(0)
    x = rng.standard_normal((B, C, H, W), dtype=np.float32)
    y = rng.standard_normal((B, C, H, W), dtype=np.float32)
    out = kernel(x=x, y=y, max_displacement=4)
    print("kernel ran, out shape", out.shape, out.dtype)
